# revision 33
# baseline (speedup 1.0000x reference)
"""GQA attention (32 q-heads, 8 kv-heads, d=128, s=2048) on 8 trn2 cores.

Sharding: one kv-head + its 4 q-heads per core (pure head-parallel, no
cross-core communication). The host pre-transposes q/k during sharding so
the device needs no on-chip transposes.

Device algorithm per core:
  scoresT[kj, qi] = kT_tile.T @ qT         (PE bf16, stationary = kT tile)
  probsT = exp(scoresT * 1/sqrt(d))        (ACT, scale fused into exp,
                                            bf16 out; fp32 PSUM in)
  out[qi, 0:129] += probsT_tile.T @ [1|v]  (PE bf16; col 0 accumulates the
                                            softmax row-sum, cols 1..128 P@V,
                                            fp32 PSUM accumulation)
  out[qi, d] = out[qi, 1+d] * 1/out[qi, 0] (DVE reciprocal + tensor_scalar)

No max-subtraction: scaled scores are ~N(0,1) (|x| < ~10), so exp is safely
in fp32 range; matches jax softmax closely (measured 3.6e-3 absmax-relative
vs the fp32 reference, dominated by the bf16 q/k and probs rounding; all
accumulations and the softmax division stay fp32).
The additive mask is all-zeros by construction in this problem; if a nonzero
mask ever shows up we fall back to an exact host computation.

Measured on HW: ~176us per core end-to-end (exp on the scalar engine is the
~142us floor, running near back-to-back; ~25us is fixed Tile prologue/exit
overhead; the rest is ramp-in and small per-iteration semaphore slack).
"""

import numpy as np

SEQ = 2048
NH = 32
NKV = 8
HD = 128
HPC = NH // NKV  # q heads per core (= per kv head)
NCORES = 8
SCALE = 1.0 / float(np.sqrt(np.float32(HD)))

_BASS = None


def _build():
    from contextlib import ExitStack

    import concourse.tile as tile
    from concourse import bacc, mybir

    f32 = mybir.dt.float32
    # float32r = same fp32 bits, but the PE runs the matmul as a single
    # reduced-precision pass (~2 cycles/row measured) instead of fp32's two
    # half-speed passes (4 cycles/row).
    f32r = mybir.dt.float32r
    bf16 = mybir.dt.bfloat16
    # Bacc (not bare Bass): its compile() pass splits >1-wait matmuls via
    # event semaphores, which walrus requires.
    nc = bacc.Bacc(None)
    qT = nc.declare_dram_parameter("qT", [HPC * HD, SEQ], bf16, isOutput=False)
    kT = nc.declare_dram_parameter("kT", [HD, SEQ], bf16, isOutput=False)
    # v arrives with a leading all-ones column: PV matmuls against [1|v]
    # accumulate the softmax row-sum in output column 0 for free, and a
    # host-built ones column keeps each matmul at <=2 sync waits (the
    # Matmult/LDWEIGHTS wait-slot limit walrus enforces). bf16: the PV
    # matmul's moving free dim is only 129, where fp32/fp32r run at 1/4 rate.
    vv = nc.declare_dram_parameter("v", [SEQ, HD + 1], bf16, isOutput=False)
    oo = nc.declare_dram_parameter("o", [HPC, SEQ, HD], f32, isOutput=True)

    NKJ = SEQ // 128  # 16 key tiles
    QCH = 1024  # qi chunk: 2 matmul chunks, one [128,1024] exp per key tile
    NCHUNK = SEQ // QCH
    NSUB = QCH // 128  # qi sub-tiles (PV accumulator groups) per chunk
    EXP = mybir.ActivationFunctionType.Exp

    with tile.TileContext(nc) as tc, ExitStack() as ctx:
        const = ctx.enter_context(tc.tile_pool(name="const", bufs=1))
        sT_pool = ctx.enter_context(tc.tile_pool(name="sT", bufs=2, space="PSUM"))
        po_pool = ctx.enter_context(tc.tile_pool(name="po", bufs=1, space="PSUM"))
        pT_pool = ctx.enter_context(tc.tile_pool(name="pT", bufs=6))
        o_pool = ctx.enter_context(tc.tile_pool(name="osb", bufs=4))
        r_pool = ctx.enter_context(tc.tile_pool(name="recip", bufs=8))
        e_pool = ctx.enter_context(tc.tile_pool(name="evac", bufs=3))

        # Preloads are split to slice granularity and emitted in first-use
        # order (DMAs drain roughly in emission order, and 9.5MB takes ~25us
        # at full fabric rate): the first key tile, the first q chunk and the
        # v tiles land within ~2us so compute starts immediately; the
        # remaining q chunks stream in well ahead of their first use.
        qT_sb = [
            const.tile([128, SEQ], bf16, tag=f"qT{h}", name=f"qTsb{h}")
            for h in range(HPC)
        ]
        kT_sb = const.tile([128, SEQ], bf16, tag="kT")
        v_aug = [
            const.tile([128, HD + 1], bf16, tag=f"vaug{j}", name=f"vaug{j}")
            for j in range(NKJ)
        ]

        # Warm the ACT table set before the first real exp: the one-time
        # ~1.3us ACT_TABLE_LOAD otherwise fires lazily at the first
        # ACTIVATE, right on the critical path after the preload DMAs.
        warm_in = const.tile([128, 1], f32, tag="warm_in", name="warm_in")
        nc.vector.memset(warm_in[:], 0.0)
        warm_out = const.tile([128, 1], f32, tag="warm_out", name="warm_out")
        nc.scalar.activation(warm_out[:], warm_in[:], EXP)

        def load_kt(j):
            nc.sync.dma_start(
                kT_sb[:, j * 128 : (j + 1) * 128], kT[:, j * 128 : (j + 1) * 128]
            )

        def load_qt(h, ci):
            nc.sync.dma_start(
                qT_sb[h][:, ci * QCH : (ci + 1) * QCH],
                qT[h * 128 : (h + 1) * 128, ci * QCH : (ci + 1) * QCH],
            )

        load_kt(0)
        # first q chunk in two halves: the first QK matmul only reads
        # columns 0:512, so it can launch after 128KB instead of 256KB
        nc.sync.dma_start(qT_sb[0][:, 0:512], qT[0:128, 0:512])
        nc.sync.dma_start(qT_sb[0][:, 512:QCH], qT[0:128, 512:QCH])
        for j in range(NKJ):
            nc.sync.dma_start(v_aug[j][:], vv[j * 128 : (j + 1) * 128, :])
            if j > 0:
                load_kt(j)
        for h in range(HPC):
            for ci in range(NCHUNK):
                if (h, ci) != (0, 0):
                    load_qt(h, ci)

        # Software-pipelined emission over the flat (head, chunk, key-tile)
        # space: QK for iteration t+1 is emitted BEFORE PV of iteration t, so
        # the in-order PE stream never sits behind exp(t+1) — while ACT runs
        # exp(t), PE does QK(t+1); when exp(t) lands, PE does PV(t). This
        # keeps both engines back-to-back (and the PE free of the idle gaps
        # that re-throttle the HAM clock gate).
        iters = [
            (h, ci, j)
            for h in range(HPC)
            for ci in range(NCHUNK)
            for j in range(NKJ)
        ]
        po_all = {}

        def emit_qk(h, ci, j):
            sT = sT_pool.tile([128, QCH], f32, tag="sT", name="sT")
            q_sl = qT_sb[h][:, ci * QCH : (ci + 1) * QCH]
            for half in range(QCH // 512):
                nc.tensor.matmul(
                    sT[:, half * 512 : (half + 1) * 512],
                    kT_sb[:, j * 128 : (j + 1) * 128],
                    q_sl[:, half * 512 : (half + 1) * 512],
                    start=True,
                    stop=True,
                )
            return sT

        sT_cur = emit_qk(*iters[0])
        for t, (h, ci, j) in enumerate(iters):
            if j == 0:
                # Two PV accumulator groups packed per PSUM bank: the s%2==0
                # group opens with start=True, which clears has_written for
                # the WHOLE bank, so its s%2==1 sibling keeps start=False
                # even on its first matmul (cleared bits make that first
                # write an overwrite, per-element).
                po_all[(h, ci)] = [
                    po_pool.tile([128, 2, HD + 1], f32, tag=f"po{b}", name=f"po{b}")
                    for b in range(NSUB // 2)
                ]
            po = po_all[(h, ci)]
            pT = pT_pool.tile([128, QCH], bf16, tag="pT", name="pT")
            nc.scalar.activation(pT[:], sT_cur[:], EXP, scale=SCALE)

            def emit_pv(s):
                nc.tensor.matmul(
                    po[s // 2][:, s % 2, :],
                    pT[:, s * 128 : (s + 1) * 128],
                    v_aug[j][:],
                    start=(j == 0 and s % 2 == 0),
                    stop=(j == NKJ - 1),
                    skip_group_check=True,
                )

            # QK(t+1) is emitted after only TWO of PV(t)'s eight matmuls:
            # exp(t+1) waits on QK(t+1) completing through a PE-sem event
            # semaphore, so QK(t+1) must finish well before exp(t) ends or
            # the ~100ns sem latency lands on the ACT critical path. Two PV
            # matmuls (~114ns) in front satisfy the exp(t)->PV(t) data dep
            # without pushing QK(t+1) late. At a chunk start (j==0) the PV
            # matmuls additionally wait on the previous chunk's PSUM
            # evacuation, so there QK(t+1) goes first.
            pre = 0 if j == 0 else 2
            for s in range(pre):
                emit_pv(s)
            if t + 1 < len(iters):
                sT_cur = emit_qk(*iters[t + 1])
            for s in range(pre, NSUB):
                emit_pv(s)
            if j == NKJ - 1:
                # Evacuate the po banks with fast raw copies so the next
                # chunk's PV accumulation can reuse them immediately; the
                # reciprocal + divide then run from SBUF off the critical
                # path.
                for b in range(NSUB // 2):
                    ev = e_pool.tile([128, 2, HD + 1], f32, tag=f"ev{b}", name=f"ev{b}")
                    nc.vector.tensor_copy(ev[:], po[b][:])
                    for sub in range(2):
                        s = b * 2 + sub
                        rec = r_pool.tile([128, 1], f32, tag="rec", name="rec")
                        nc.vector.reciprocal(rec[:], ev[:, sub, 0:1])
                        osb = o_pool.tile([128, HD], f32, tag="osb", name="osb")
                        nc.vector.tensor_scalar_mul(
                            osb[:], ev[:, sub, 1 : HD + 1], rec[:]
                        )
                        r0 = ci * QCH + s * 128
                        nc.sync.dma_start(oo[h, r0 : r0 + 128, :], osb[:])
                del po_all[(h, ci)]

    nc.finalize()
    return nc


def _get_bass():
    global _BASS
    if _BASS is None:
        _BASS = _build()
    return _BASS


def _fallback(q, k, v, mask):
    # exact reference math on host, one head at a time (nonzero mask path)
    rep = NH // NKV
    out = np.empty((SEQ, NH, HD), np.float32)
    kh = k.reshape(SEQ, NKV, HD)
    vh = v.reshape(SEQ, NKV, HD)
    for g in range(NH):
        s = (q.reshape(SEQ, NH, HD)[:, g, :] @ kh[:, g // rep, :].T) * np.float32(SCALE)
        s = s + mask
        s -= s.max(axis=-1, keepdims=True)
        p = np.exp(s)
        p /= p.sum(axis=-1, keepdims=True)
        out[:, g, :] = p @ vh[:, g // rep, :]
    return out.reshape(SEQ, NH * HD)


def make_in_maps(q, k, v):
    import ml_dtypes

    qh = q.reshape(SEQ, NH, HD)
    kh = k.reshape(SEQ, NKV, HD)
    vh = v.reshape(SEQ, NKV, HD)
    in_maps = []
    for c in range(NCORES):
        qT = np.ascontiguousarray(
            qh[:, HPC * c : HPC * (c + 1), :].transpose(1, 2, 0).astype(ml_dtypes.bfloat16)
        ).reshape(HPC * HD, SEQ)
        kTc = np.ascontiguousarray(kh[:, c, :].T.astype(ml_dtypes.bfloat16))
        vc = np.empty((SEQ, HD + 1), ml_dtypes.bfloat16)
        vc[:, 0] = 1.0
        vc[:, 1:] = vh[:, c, :].astype(ml_dtypes.bfloat16)
        in_maps.append({"qT": qT, "kT": kTc, "v": vc})
    return in_maps


def kernel(q, k, v, mask):
    q = np.ascontiguousarray(np.asarray(q, dtype=np.float32))
    k = np.ascontiguousarray(np.asarray(k, dtype=np.float32))
    v = np.ascontiguousarray(np.asarray(v, dtype=np.float32))
    mask = np.asarray(mask, dtype=np.float32)
    if mask.any():
        return _fallback(q, k, v, mask)

    nc = _get_bass()
    in_maps = make_in_maps(q, k, v)

    from concourse.bass_utils import run_bass_kernel_spmd

    res = run_bass_kernel_spmd(nc, in_maps, list(range(NCORES)))
    out = np.empty((SEQ, NH, HD), np.float32)
    for c in range(NCORES):
        oc = np.asarray(res.results[c]["o"])  # [HPC, SEQ, HD]
        out[:, HPC * c : HPC * (c + 1), :] = oc.transpose(1, 0, 2)
    return out.reshape(SEQ, NH * HD)


# revision 34
# speedup vs baseline: 1.1865x; 1.1865x over previous
"""GQA attention (32 q-heads, 8 kv-heads, d=128, s=2048) on 8 trn2 cores.

Sharding: one kv-head + its 4 q-heads per core (pure head-parallel, no
cross-core communication). The host pre-transposes q/k during sharding so
the device needs no on-chip transposes.

Device algorithm per core:
  scoresT[kj, qi] = kT_tile.T @ qT         (PE bf16, stationary = kT tile)
  probsT = exp(scoresT * 1/sqrt(d))        (ACT, scale fused into exp,
                                            bf16 out; fp32 PSUM in)
  out[qi, 0:129] += probsT_tile.T @ [1|v]  (PE bf16; col 0 accumulates the
                                            softmax row-sum, cols 1..128 P@V,
                                            fp32 PSUM accumulation)
  out[qi, d] = out[qi, 1+d] * 1/out[qi, 0] (DVE reciprocal + tensor_scalar)

No max-subtraction: scaled scores are ~N(0,1) (|x| < ~10), so exp is safely
in fp32 range; matches jax softmax closely (measured 3.6e-3 absmax-relative
vs the fp32 reference, dominated by the bf16 q/k and probs rounding; all
accumulations and the softmax division stay fp32).
The additive mask is all-zeros by construction in this problem; if a nonzero
mask ever shows up we fall back to an exact host computation.

Measured on HW: ~176us per core end-to-end (exp on the scalar engine is the
~142us floor, running near back-to-back; ~25us is fixed Tile prologue/exit
overhead; the rest is ramp-in and small per-iteration semaphore slack).
"""

import numpy as np

SEQ = 2048
NH = 32
NKV = 8
HD = 128
HPC = NH // NKV  # q heads per core (= per kv head)
NCORES = 8
SCALE = 1.0 / float(np.sqrt(np.float32(HD)))

_BASS = None


def _build():
    from contextlib import ExitStack

    import concourse.tile as tile
    from concourse import bacc, mybir

    f32 = mybir.dt.float32
    # float32r = same fp32 bits, but the PE runs the matmul as a single
    # reduced-precision pass (~2 cycles/row measured) instead of fp32's two
    # half-speed passes (4 cycles/row).
    f32r = mybir.dt.float32r
    bf16 = mybir.dt.bfloat16
    # Bacc (not bare Bass): its compile() pass splits >1-wait matmuls via
    # event semaphores, which walrus requires.
    nc = bacc.Bacc(None)
    qT = nc.declare_dram_parameter("qT", [HPC * HD, SEQ], bf16, isOutput=False)
    kT = nc.declare_dram_parameter("kT", [HD, SEQ], bf16, isOutput=False)
    # v arrives with a leading all-ones column: PV matmuls against [1|v]
    # accumulate the softmax row-sum in output column 0 for free, and a
    # host-built ones column keeps each matmul at <=2 sync waits (the
    # Matmult/LDWEIGHTS wait-slot limit walrus enforces). bf16: the PV
    # matmul's moving free dim is only 129, where fp32/fp32r run at 1/4 rate.
    vv = nc.declare_dram_parameter("v", [SEQ, HD + 1], bf16, isOutput=False)
    oo = nc.declare_dram_parameter("o", [HPC, SEQ, HD], f32, isOutput=True)

    NKJ = SEQ // 128  # 16 key tiles
    QCH = 1024  # qi chunk: 2 matmul chunks, one [128,1024] exp per key tile
    NCHUNK = SEQ // QCH
    NSUB = QCH // 128  # qi sub-tiles (PV accumulator groups) per chunk
    EXP = mybir.ActivationFunctionType.Exp

    with tile.TileContext(nc) as tc, ExitStack() as ctx:
        const = ctx.enter_context(tc.tile_pool(name="const", bufs=1))
        sT_pool = ctx.enter_context(tc.tile_pool(name="sT", bufs=2, space="PSUM"))
        po_pool = ctx.enter_context(tc.tile_pool(name="po", bufs=1, space="PSUM"))
        pT_pool = ctx.enter_context(tc.tile_pool(name="pT", bufs=6))
        o_pool = ctx.enter_context(tc.tile_pool(name="osb", bufs=4))
        r_pool = ctx.enter_context(tc.tile_pool(name="recip", bufs=8))
        e_pool = ctx.enter_context(tc.tile_pool(name="evac", bufs=3))

        # Preloads are split to slice granularity and emitted in first-use
        # order (DMAs drain roughly in emission order, and 9.5MB takes ~25us
        # at full fabric rate): the first key tile, the first q chunk and the
        # v tiles land within ~2us so compute starts immediately; the
        # remaining q chunks stream in well ahead of their first use.
        qT_sb = [
            const.tile([128, SEQ], bf16, tag=f"qT{h}", name=f"qTsb{h}")
            for h in range(HPC)
        ]
        kT_sb = const.tile([128, SEQ], bf16, tag="kT")
        v_aug = [
            const.tile([128, HD + 1], bf16, tag=f"vaug{j}", name=f"vaug{j}")
            for j in range(NKJ)
        ]

        def load_kt(j):
            nc.sync.dma_start(
                kT_sb[:, j * 128 : (j + 1) * 128], kT[:, j * 128 : (j + 1) * 128]
            )

        def load_qt(h, ci):
            nc.sync.dma_start(
                qT_sb[h][:, ci * QCH : (ci + 1) * QCH],
                qT[h * 128 : (h + 1) * 128, ci * QCH : (ci + 1) * QCH],
            )

        load_kt(0)
        load_qt(0, 0)
        for j in range(NKJ):
            nc.sync.dma_start(v_aug[j][:], vv[j * 128 : (j + 1) * 128, :])
            if j > 0:
                load_kt(j)
        for h in range(HPC):
            for ci in range(NCHUNK):
                if (h, ci) != (0, 0):
                    load_qt(h, ci)

        # Software-pipelined emission over the flat (head, chunk, key-tile)
        # space: QK for iteration t+1 is emitted BEFORE PV of iteration t, so
        # the in-order PE stream never sits behind exp(t+1) — while ACT runs
        # exp(t), PE does QK(t+1); when exp(t) lands, PE does PV(t). This
        # keeps both engines back-to-back (and the PE free of the idle gaps
        # that re-throttle the HAM clock gate).
        iters = [
            (h, ci, j)
            for h in range(HPC)
            for ci in range(NCHUNK)
            for j in range(NKJ)
        ]
        po_all = {}

        def emit_qk(h, ci, j):
            sT = sT_pool.tile([128, QCH], f32, tag="sT", name="sT")
            q_sl = qT_sb[h][:, ci * QCH : (ci + 1) * QCH]
            for half in range(QCH // 512):
                nc.tensor.matmul(
                    sT[:, half * 512 : (half + 1) * 512],
                    kT_sb[:, j * 128 : (j + 1) * 128],
                    q_sl[:, half * 512 : (half + 1) * 512],
                    start=True,
                    stop=True,
                )
            return sT

        sT_cur = emit_qk(*iters[0])
        for t, (h, ci, j) in enumerate(iters):
            if j == 0:
                # Two PV accumulator groups packed per PSUM bank: the s%2==0
                # group opens with start=True, which clears has_written for
                # the WHOLE bank, so its s%2==1 sibling keeps start=False
                # even on its first matmul (cleared bits make that first
                # write an overwrite, per-element).
                po_all[(h, ci)] = [
                    po_pool.tile([128, 2, HD + 1], f32, tag=f"po{b}", name=f"po{b}")
                    for b in range(NSUB // 2)
                ]
            po = po_all[(h, ci)]
            pT = pT_pool.tile([128, QCH], bf16, tag="pT", name="pT")
            nc.scalar.activation(pT[:], sT_cur[:], EXP, scale=SCALE)

            def emit_pv(s):
                nc.tensor.matmul(
                    po[s // 2][:, s % 2, :],
                    pT[:, s * 128 : (s + 1) * 128],
                    v_aug[j][:],
                    start=(j == 0 and s % 2 == 0),
                    stop=(j == NKJ - 1),
                    skip_group_check=True,
                )

            # QK(t+1) is emitted after only TWO of PV(t)'s eight matmuls:
            # exp(t+1) waits on QK(t+1) completing through a PE-sem event
            # semaphore, so QK(t+1) must finish well before exp(t) ends or
            # the ~100ns sem latency lands on the ACT critical path. Two PV
            # matmuls (~114ns) in front satisfy the exp(t)->PV(t) data dep
            # without pushing QK(t+1) late. At a chunk start (j==0) the PV
            # matmuls additionally wait on the previous chunk's PSUM
            # evacuation, so there QK(t+1) goes first.
            pre = 0 if j == 0 else 2
            for s in range(pre):
                emit_pv(s)
            if t + 1 < len(iters):
                sT_cur = emit_qk(*iters[t + 1])
            for s in range(pre, NSUB):
                emit_pv(s)
            if j == NKJ - 1:
                # Evacuate the po banks with fast raw copies so the next
                # chunk's PV accumulation can reuse them immediately; the
                # reciprocal + divide then run from SBUF off the critical
                # path.
                for b in range(NSUB // 2):
                    ev = e_pool.tile([128, 2, HD + 1], f32, tag=f"ev{b}", name=f"ev{b}")
                    nc.vector.tensor_copy(ev[:], po[b][:])
                    for sub in range(2):
                        s = b * 2 + sub
                        rec = r_pool.tile([128, 1], f32, tag="rec", name="rec")
                        nc.vector.reciprocal(rec[:], ev[:, sub, 0:1])
                        osb = o_pool.tile([128, HD], f32, tag="osb", name="osb")
                        nc.vector.tensor_scalar_mul(
                            osb[:], ev[:, sub, 1 : HD + 1], rec[:]
                        )
                        r0 = ci * QCH + s * 128
                        nc.sync.dma_start(oo[h, r0 : r0 + 128, :], osb[:])
                del po_all[(h, ci)]

    nc.finalize()
    return nc


def _get_bass():
    global _BASS
    if _BASS is None:
        _BASS = _build()
    return _BASS


def _fallback(q, k, v, mask):
    # exact reference math on host, one head at a time (nonzero mask path)
    rep = NH // NKV
    out = np.empty((SEQ, NH, HD), np.float32)
    kh = k.reshape(SEQ, NKV, HD)
    vh = v.reshape(SEQ, NKV, HD)
    for g in range(NH):
        s = (q.reshape(SEQ, NH, HD)[:, g, :] @ kh[:, g // rep, :].T) * np.float32(SCALE)
        s = s + mask
        s -= s.max(axis=-1, keepdims=True)
        p = np.exp(s)
        p /= p.sum(axis=-1, keepdims=True)
        out[:, g, :] = p @ vh[:, g // rep, :]
    return out.reshape(SEQ, NH * HD)


def make_in_maps(q, k, v):
    import ml_dtypes

    qh = q.reshape(SEQ, NH, HD)
    kh = k.reshape(SEQ, NKV, HD)
    vh = v.reshape(SEQ, NKV, HD)
    in_maps = []
    for c in range(NCORES):
        qT = np.ascontiguousarray(
            qh[:, HPC * c : HPC * (c + 1), :].transpose(1, 2, 0).astype(ml_dtypes.bfloat16)
        ).reshape(HPC * HD, SEQ)
        kTc = np.ascontiguousarray(kh[:, c, :].T.astype(ml_dtypes.bfloat16))
        vc = np.empty((SEQ, HD + 1), ml_dtypes.bfloat16)
        vc[:, 0] = 1.0
        vc[:, 1:] = vh[:, c, :].astype(ml_dtypes.bfloat16)
        in_maps.append({"qT": qT, "kT": kTc, "v": vc})
    return in_maps


def kernel(q, k, v, mask):
    q = np.ascontiguousarray(np.asarray(q, dtype=np.float32))
    k = np.ascontiguousarray(np.asarray(k, dtype=np.float32))
    v = np.ascontiguousarray(np.asarray(v, dtype=np.float32))
    mask = np.asarray(mask, dtype=np.float32)
    if mask.any():
        return _fallback(q, k, v, mask)

    nc = _get_bass()
    in_maps = make_in_maps(q, k, v)

    from concourse.bass_utils import run_bass_kernel_spmd

    res = run_bass_kernel_spmd(nc, in_maps, list(range(NCORES)))
    out = np.empty((SEQ, NH, HD), np.float32)
    for c in range(NCORES):
        oc = np.asarray(res.results[c]["o"])  # [HPC, SEQ, HD]
        out[:, HPC * c : HPC * (c + 1), :] = oc.transpose(1, 0, 2)
    return out.reshape(SEQ, NH * HD)


# revision 35
# speedup vs baseline: 1.1955x; 1.0076x over previous
"""GQA attention (32 q-heads, 8 kv-heads, d=128, s=2048) on 8 trn2 cores.

Sharding: one kv-head + its 4 q-heads per core (pure head-parallel, no
cross-core communication). The host pre-transposes q/k during sharding so
the device needs no on-chip transposes.

Device algorithm per core:
  scoresT[kj, qi] = kT_tile.T @ qT         (PE bf16, stationary = kT tile)
  probsT = exp(scoresT * 1/sqrt(d))        (ACT, scale fused into exp,
                                            bf16 out; fp32 PSUM in)
  out[qi, 0:129] += probsT_tile.T @ [1|v]  (PE bf16; col 0 accumulates the
                                            softmax row-sum, cols 1..128 P@V,
                                            fp32 PSUM accumulation)
  out[qi, d] = out[qi, 1+d] * 1/out[qi, 0] (DVE reciprocal + tensor_scalar)

No max-subtraction: scaled scores are ~N(0,1) (|x| < ~10), so exp is safely
in fp32 range; matches jax softmax closely (measured 3.6e-3 absmax-relative
vs the fp32 reference, dominated by the bf16 q/k and probs rounding; all
accumulations and the softmax division stay fp32).
The additive mask is all-zeros by construction in this problem; if a nonzero
mask ever shows up we fall back to an exact host computation.

Measured on HW: ~176us per core end-to-end (exp on the scalar engine is the
~142us floor, running near back-to-back; ~25us is fixed Tile prologue/exit
overhead; the rest is ramp-in and small per-iteration semaphore slack).
"""

import numpy as np

SEQ = 2048
NH = 32
NKV = 8
HD = 128
HPC = NH // NKV  # q heads per core (= per kv head)
NCORES = 8
SCALE = 1.0 / float(np.sqrt(np.float32(HD)))

_BASS = None


def _build():
    from contextlib import ExitStack

    import concourse.tile as tile
    from concourse import bacc, mybir

    f32 = mybir.dt.float32
    # float32r = same fp32 bits, but the PE runs the matmul as a single
    # reduced-precision pass (~2 cycles/row measured) instead of fp32's two
    # half-speed passes (4 cycles/row).
    f32r = mybir.dt.float32r
    bf16 = mybir.dt.bfloat16
    # Bacc (not bare Bass): its compile() pass splits >1-wait matmuls via
    # event semaphores, which walrus requires.
    nc = bacc.Bacc(None)
    qT = nc.declare_dram_parameter("qT", [HPC * HD, SEQ], bf16, isOutput=False)
    kT = nc.declare_dram_parameter("kT", [HD, SEQ], bf16, isOutput=False)
    # v arrives with a leading all-ones column: PV matmuls against [1|v]
    # accumulate the softmax row-sum in output column 0 for free, and a
    # host-built ones column keeps each matmul at <=2 sync waits (the
    # Matmult/LDWEIGHTS wait-slot limit walrus enforces). bf16: the PV
    # matmul's moving free dim is only 129, where fp32/fp32r run at 1/4 rate.
    vv = nc.declare_dram_parameter("v", [SEQ, HD + 1], bf16, isOutput=False)
    oo = nc.declare_dram_parameter("o", [HPC, SEQ, HD], f32, isOutput=True)

    NKJ = SEQ // 128  # 16 key tiles
    QCH = 1024  # qi chunk: 2 matmul chunks, one [128,1024] exp per key tile
    NCHUNK = SEQ // QCH
    NSUB = QCH // 128  # qi sub-tiles (PV accumulator groups) per chunk
    EXP = mybir.ActivationFunctionType.Exp

    with tile.TileContext(nc) as tc, ExitStack() as ctx:
        const = ctx.enter_context(tc.tile_pool(name="const", bufs=1))
        sT_pool = ctx.enter_context(tc.tile_pool(name="sT", bufs=2, space="PSUM"))
        po_pool = ctx.enter_context(tc.tile_pool(name="po", bufs=1, space="PSUM"))
        pT_pool = ctx.enter_context(tc.tile_pool(name="pT", bufs=6))
        o_pool = ctx.enter_context(tc.tile_pool(name="osb", bufs=4))
        r_pool = ctx.enter_context(tc.tile_pool(name="recip", bufs=8))
        e_pool = ctx.enter_context(tc.tile_pool(name="evac", bufs=3))

        # Preloads are split to slice granularity and emitted in first-use
        # order (DMAs drain roughly in emission order, and 9.5MB takes ~25us
        # at full fabric rate): the first key tile, the first q chunk and the
        # v tiles land within ~2us so compute starts immediately; the
        # remaining q chunks stream in well ahead of their first use.
        qT_sb = [
            const.tile([128, SEQ], bf16, tag=f"qT{h}", name=f"qTsb{h}")
            for h in range(HPC)
        ]
        kT_sb = const.tile([128, SEQ], bf16, tag="kT")
        v_aug = [
            const.tile([128, HD + 1], bf16, tag=f"vaug{j}", name=f"vaug{j}")
            for j in range(NKJ)
        ]

        def load_kt(j):
            nc.sync.dma_start(
                kT_sb[:, j * 128 : (j + 1) * 128], kT[:, j * 128 : (j + 1) * 128]
            )

        def load_qt(h, ci):
            nc.sync.dma_start(
                qT_sb[h][:, ci * QCH : (ci + 1) * QCH],
                qT[h * 128 : (h + 1) * 128, ci * QCH : (ci + 1) * QCH],
            )

        load_kt(0)
        load_qt(0, 0)
        for j in range(NKJ):
            nc.sync.dma_start(v_aug[j][:], vv[j * 128 : (j + 1) * 128, :])
            if j > 0:
                load_kt(j)
        for h in range(HPC):
            for ci in range(NCHUNK):
                if (h, ci) != (0, 0):
                    load_qt(h, ci)

        # Software-pipelined emission over the flat (head, chunk, key-tile)
        # space: QK for iteration t+1 is emitted BEFORE PV of iteration t, so
        # the in-order PE stream never sits behind exp(t+1) — while ACT runs
        # exp(t), PE does QK(t+1); when exp(t) lands, PE does PV(t). This
        # keeps both engines back-to-back (and the PE free of the idle gaps
        # that re-throttle the HAM clock gate).
        iters = [
            (h, ci, j)
            for h in range(HPC)
            for ci in range(NCHUNK)
            for j in range(NKJ)
        ]
        po_all = {}

        def emit_qk(h, ci, j):
            sT = sT_pool.tile([128, QCH], f32, tag="sT", name="sT")
            q_sl = qT_sb[h][:, ci * QCH : (ci + 1) * QCH]
            for half in range(QCH // 512):
                nc.tensor.matmul(
                    sT[:, half * 512 : (half + 1) * 512],
                    kT_sb[:, j * 128 : (j + 1) * 128],
                    q_sl[:, half * 512 : (half + 1) * 512],
                    start=True,
                    stop=True,
                )
            return sT

        sT_cur = emit_qk(*iters[0])
        for t, (h, ci, j) in enumerate(iters):
            if j == 0:
                # Two PV accumulator groups packed per PSUM bank: the s%2==0
                # group opens with start=True, which clears has_written for
                # the WHOLE bank, so its s%2==1 sibling keeps start=False
                # even on its first matmul (cleared bits make that first
                # write an overwrite, per-element).
                po_all[(h, ci)] = [
                    po_pool.tile([128, 2, HD + 1], f32, tag=f"po{b}", name=f"po{b}")
                    for b in range(NSUB // 2)
                ]
            po = po_all[(h, ci)]
            pT = pT_pool.tile([128, QCH], bf16, tag="pT", name="pT")
            nc.scalar.activation(pT[:], sT_cur[:], EXP, scale=SCALE)

            def emit_pv(s):
                nc.tensor.matmul(
                    po[s // 2][:, s % 2, :],
                    pT[:, s * 128 : (s + 1) * 128],
                    v_aug[j][:],
                    start=(j == 0 and s % 2 == 0),
                    stop=(j == NKJ - 1),
                    skip_group_check=True,
                )

            # QK(t+1) is emitted after only TWO of PV(t)'s eight matmuls:
            # exp(t+1) waits on QK(t+1) completing through a PE-sem event
            # semaphore, so QK(t+1) must finish well before exp(t) ends or
            # the ~100ns sem latency lands on the ACT critical path. Two PV
            # matmuls (~114ns) in front satisfy the exp(t)->PV(t) data dep
            # without pushing QK(t+1) late. At a chunk start (j==0) the PV
            # matmuls additionally wait on the previous chunk's PSUM
            # evacuation, so there QK(t+1) goes first.
            pre = 0 if j == 0 else 2
            evs = []

            def emit_pv_and_evac(s):
                emit_pv(s)
                # On the last key tile, po[s//2]'s final write is matmul
                # s=2b+1 — evacuate that bank immediately (fast raw copy)
                # instead of after the whole PV loop, so the next chunk's
                # accumulation reuses the banks ~1us earlier.
                if j == NKJ - 1 and s % 2 == 1:
                    b = s // 2
                    ev = e_pool.tile(
                        [128, 2, HD + 1], f32, tag=f"ev{b}", name=f"ev{b}"
                    )
                    nc.vector.tensor_copy(ev[:], po[b][:])
                    evs.append(ev)

            for s in range(pre):
                emit_pv_and_evac(s)
            if t + 1 < len(iters):
                sT_cur = emit_qk(*iters[t + 1])
            for s in range(pre, NSUB):
                emit_pv_and_evac(s)
            if j == NKJ - 1:
                # reciprocal + divide run from the SBUF copies, off the
                # critical path.
                for b in range(NSUB // 2):
                    ev = evs[b]
                    for sub in range(2):
                        s = b * 2 + sub
                        rec = r_pool.tile([128, 1], f32, tag="rec", name="rec")
                        nc.vector.reciprocal(rec[:], ev[:, sub, 0:1])
                        osb = o_pool.tile([128, HD], f32, tag="osb", name="osb")
                        nc.vector.tensor_scalar_mul(
                            osb[:], ev[:, sub, 1 : HD + 1], rec[:]
                        )
                        r0 = ci * QCH + s * 128
                        nc.sync.dma_start(oo[h, r0 : r0 + 128, :], osb[:])
                del po_all[(h, ci)]

    nc.finalize()
    return nc


def _get_bass():
    global _BASS
    if _BASS is None:
        _BASS = _build()
    return _BASS


def _fallback(q, k, v, mask):
    # exact reference math on host, one head at a time (nonzero mask path)
    rep = NH // NKV
    out = np.empty((SEQ, NH, HD), np.float32)
    kh = k.reshape(SEQ, NKV, HD)
    vh = v.reshape(SEQ, NKV, HD)
    for g in range(NH):
        s = (q.reshape(SEQ, NH, HD)[:, g, :] @ kh[:, g // rep, :].T) * np.float32(SCALE)
        s = s + mask
        s -= s.max(axis=-1, keepdims=True)
        p = np.exp(s)
        p /= p.sum(axis=-1, keepdims=True)
        out[:, g, :] = p @ vh[:, g // rep, :]
    return out.reshape(SEQ, NH * HD)


def make_in_maps(q, k, v):
    import ml_dtypes

    qh = q.reshape(SEQ, NH, HD)
    kh = k.reshape(SEQ, NKV, HD)
    vh = v.reshape(SEQ, NKV, HD)
    in_maps = []
    for c in range(NCORES):
        qT = np.ascontiguousarray(
            qh[:, HPC * c : HPC * (c + 1), :].transpose(1, 2, 0).astype(ml_dtypes.bfloat16)
        ).reshape(HPC * HD, SEQ)
        kTc = np.ascontiguousarray(kh[:, c, :].T.astype(ml_dtypes.bfloat16))
        vc = np.empty((SEQ, HD + 1), ml_dtypes.bfloat16)
        vc[:, 0] = 1.0
        vc[:, 1:] = vh[:, c, :].astype(ml_dtypes.bfloat16)
        in_maps.append({"qT": qT, "kT": kTc, "v": vc})
    return in_maps


def kernel(q, k, v, mask):
    q = np.ascontiguousarray(np.asarray(q, dtype=np.float32))
    k = np.ascontiguousarray(np.asarray(k, dtype=np.float32))
    v = np.ascontiguousarray(np.asarray(v, dtype=np.float32))
    mask = np.asarray(mask, dtype=np.float32)
    if mask.any():
        return _fallback(q, k, v, mask)

    nc = _get_bass()
    in_maps = make_in_maps(q, k, v)

    from concourse.bass_utils import run_bass_kernel_spmd

    res = run_bass_kernel_spmd(nc, in_maps, list(range(NCORES)))
    out = np.empty((SEQ, NH, HD), np.float32)
    for c in range(NCORES):
        oc = np.asarray(res.results[c]["o"])  # [HPC, SEQ, HD]
        out[:, HPC * c : HPC * (c + 1), :] = oc.transpose(1, 0, 2)
    return out.reshape(SEQ, NH * HD)
